# revision 5
# baseline (speedup 1.0000x reference)
"""CoLA encoder layer on 8 Trainium2 NeuronCores, data-parallel over batch.

Per core (one batch element, L=4096 tokens, D=1024, ALPHA=64):
  Q^T = W_Q^T-major matmul (bf16), S = Q@C_K (bf16), softmax+mask,
  attn = A@C_V^T (fp32r), LayerNorm residual (fp32), pointwise conv
  (fp32r) + LeakyReLU, second LayerNorm residual. Token-major layout for
  LayerNorms/softmax; feature-major (transposed) operands for matmuls,
  produced on host (x^T) or on chip (z^T via PE transpose).
"""

import sys

for _p in ("/opt/trn_rl_repo",):
    if _p not in sys.path:
        sys.path.insert(0, _p)

from contextlib import ExitStack

import ml_dtypes
import numpy as np

import concourse.bass as bass
import concourse.tile as tile
from concourse import bacc, mybir
from concourse.bass_utils import run_bass_kernel_spmd

F32 = mybir.dt.float32
F32R = mybir.dt.float32r
BF16 = mybir.dt.bfloat16
AF = mybir.ActivationFunctionType
ALU = mybir.AluOpType
AX = mybir.AxisListType

B, L, D, ALPHA = 8, 4096, 1024, 64
EPS = 1e-6
INV_SQRT_D = 1.0 / 32.0
CH = 512           # tokens per chunk
NCH = L // CH      # 8 chunks
LTPC = CH // 128   # l-tiles per chunk
DT = D // 128      # feature tiles

_CACHE = {}


def _build(g1_trivial: bool, g2_trivial: bool, time_iters: int = 1):
    nc = bacc.Bacc("TRN2", target_bir_lowering=False, debug=False)

    x_d = nc.dram_tensor("x", [L, D], F32, kind="ExternalInput")
    xt_d = nc.dram_tensor("xt", [D, L], BF16, kind="ExternalInput")
    wq_d = nc.dram_tensor("wq", [D, D], BF16, kind="ExternalInput")
    wc_d = nc.dram_tensor("wc", [D, D], F32R, kind="ExternalInput")
    ck_d = nc.dram_tensor("ck", [D, ALPHA], BF16, kind="ExternalInput")
    cv_d = nc.dram_tensor("cv", [ALPHA, D], F32R, kind="ExternalInput")
    bq_d = nc.dram_tensor("bq", [128, DT], F32, kind="ExternalInput")
    bc_d = nc.dram_tensor("bc", [1, D], F32R, kind="ExternalInput")
    mask_d = nc.dram_tensor("maskf", [128, L // 128], F32, kind="ExternalInput")
    id_d = nc.dram_tensor("ident", [128, 128], F32, kind="ExternalInput")
    ones_d = nc.dram_tensor("ones", [1, 128], F32R, kind="ExternalInput")
    if not g1_trivial:
        g1_d = nc.dram_tensor("g1r", [1, D], F32, kind="ExternalInput")
        be1_d = nc.dram_tensor("be1r", [1, D], F32, kind="ExternalInput")
    if not g2_trivial:
        g2_d = nc.dram_tensor("g2r", [1, D], F32, kind="ExternalInput")
        be2_d = nc.dram_tensor("be2r", [1, D], F32, kind="ExternalInput")
    out_d = nc.dram_tensor("out", [L, D], F32, kind="ExternalOutput")

    x_ap = x_d.ap()
    xt_ap = xt_d.ap().rearrange("(t p) l -> p t l", p=128)
    out_ap = out_d.ap()

    with tile.TileContext(nc) as tc:
        with ExitStack() as ctx:
            wp = ctx.enter_context(tc.tile_pool(name="weights", bufs=1))
            xt_pool = ctx.enter_context(tc.tile_pool(name="xtp", bufs=2))
            qt_pool = ctx.enter_context(tc.tile_pool(name="qtp", bufs=2))
            ht_pool = ctx.enter_context(tc.tile_pool(name="htp", bufs=2))
            xz_pool = ctx.enter_context(tc.tile_pool(name="xzp", bufs=8))
            ya_pool = ctx.enter_context(tc.tile_pool(name="yap", bufs=6))
            a_pool = ctx.enter_context(tc.tile_pool(name="ap", bufs=8))
            st_pool = ctx.enter_context(tc.tile_pool(name="stp", bufs=12))
            ps_mm = ctx.enter_context(tc.tile_pool(name="psmm", bufs=4, space="PSUM"))
            ps_s = ctx.enter_context(tc.tile_pool(name="pss", bufs=2, space="PSUM"))
            ps_at = ctx.enter_context(tc.tile_pool(name="psat", bufs=1, space="PSUM"))
            ps_zt = ctx.enter_context(tc.tile_pool(name="pszt", bufs=1, space="PSUM"))

            wq_sb = wp.tile([128, DT, D], BF16)
            nc.sync.dma_start(wq_sb, wq_d.ap().rearrange("(t p) e -> p t e", p=128))
            wc_sb = wp.tile([128, DT, D], F32R)
            nc.sync.dma_start(wc_sb, wc_d.ap().rearrange("(t p) e -> p t e", p=128))
            ck_sb = wp.tile([128, DT, ALPHA], BF16)
            nc.sync.dma_start(ck_sb, ck_d.ap().rearrange("(t p) a -> p t a", p=128))
            cv_sb = wp.tile([ALPHA, D], F32R)
            nc.sync.dma_start(cv_sb, cv_d.ap())
            bq_sb = wp.tile([128, DT], F32)
            nc.sync.dma_start(bq_sb, bq_d.ap())
            bc_sb = wp.tile([1, D], F32R)
            nc.sync.dma_start(bc_sb, bc_d.ap())
            mask_sb = wp.tile([128, L // 128], F32)
            nc.sync.dma_start(mask_sb, mask_d.ap())
            id_sb = wp.tile([128, 128], F32)
            nc.sync.dma_start(id_sb, id_d.ap())
            ones_sb = wp.tile([1, 128], F32R)
            nc.sync.dma_start(ones_sb, ones_d.ap())
            if not g1_trivial:
                g1_sb = wp.tile([128, D], F32)
                nc.sync.dma_start(g1_sb, g1_d.ap().to_broadcast((128, D)))
                be1_sb = wp.tile([128, D], F32)
                nc.sync.dma_start(be1_sb, be1_d.ap().to_broadcast((128, D)))
            if not g2_trivial:
                g2_sb = wp.tile([128, D], F32)
                nc.sync.dma_start(g2_sb, g2_d.ap().to_broadcast((128, D)))
                be2_sb = wp.tile([128, D], F32)
                nc.sync.dma_start(be2_sb, be2_d.ap().to_broadcast((128, D)))

            loop_cm = (
                tc.For_i(0, time_iters, 1) if time_iters > 1 else None
            )
            if loop_cm is not None:
                loop_cm.__enter__()
            for c in range(NCH):
                l0 = c * CH
                xt_sb = xt_pool.tile([128, DT, CH], BF16, tag="xt")
                nc.sync.dma_start(xt_sb, xt_ap[:, :, l0 : l0 + CH])
                xz = []
                for lt in range(LTPC):
                    t = xz_pool.tile([128, D], F32, name=f"xz{lt}", tag="xz")
                    nc.sync.dma_start(
                        t, x_ap[l0 + lt * 128 : l0 + (lt + 1) * 128, :]
                    )
                    xz.append(t)

                # Q^T[e, l] per e-tile, bf16 inputs, +b_Q on PSUM copy
                qt_sb = qt_pool.tile([128, DT, CH], BF16, tag="qt")
                for e in range(DT):
                    pq = ps_mm.tile([128, CH], F32, name="pq", tag="mm")
                    for d in range(DT):
                        nc.tensor.matmul(
                            pq,
                            wq_sb[:, d, e * 128 : (e + 1) * 128],
                            xt_sb[:, d, :],
                            start=(d == 0),
                            stop=(d == DT - 1),
                        )
                    nc.scalar.activation(
                        qt_sb[:, e, :], pq, AF.Identity, bias=bq_sb[:, e : e + 1]
                    )

                ht_sb = ht_pool.tile([128, DT, CH], F32R, tag="ht")
                for lt in range(LTPC):
                    l1 = lt * 128
                    # S[l, a] then softmax
                    ps = ps_s.tile([128, ALPHA], F32, name="ps", tag="ps")
                    for e in range(DT):
                        nc.tensor.matmul(
                            ps,
                            qt_sb[:, e, l1 : l1 + 128],
                            ck_sb[:, e, :],
                            start=(e == 0),
                            stop=(e == DT - 1),
                        )
                    mx = st_pool.tile([128, 1], F32, name="mx", tag="mx")
                    nc.vector.reduce_max(mx, ps, axis=AX.X)
                    nm = st_pool.tile([128, 1], F32, name="nm", tag="nm")
                    nc.scalar.activation(nm, mx, AF.Copy, scale=-INV_SQRT_D)
                    ev = a_pool.tile([128, ALPHA], F32, name="ev", tag="ev")
                    sm = st_pool.tile([128, 1], F32, name="sm", tag="sm")
                    nc.scalar.activation(
                        ev, ps, AF.Exp, scale=INV_SQRT_D, bias=nm, accum_out=sm
                    )
                    rc = st_pool.tile([128, 1], F32, name="rc", tag="rc")
                    nc.vector.reciprocal(rc, sm)
                    r2 = st_pool.tile([128, 1], F32, name="r2", tag="r2")
                    li = c * LTPC + lt
                    nc.vector.tensor_mul(r2, rc, mask_sb[:, li : li + 1])
                    nc.vector.tensor_scalar_mul(ev, ev, r2)
                    # A^T
                    pat = ps_at.tile([ALPHA, 128], F32, name="pat", tag="pat")
                    nc.tensor.transpose(pat, ev, id_sb)
                    at = a_pool.tile([ALPHA, 128], F32R, name="at", tag="at")
                    nc.vector.tensor_copy(at, pat)
                    # attn + residual
                    for hf in range(2):
                        pa = ps_mm.tile([128, 512], F32, name="pa", tag="mm")
                        nc.tensor.matmul(
                            pa,
                            at,
                            cv_sb[:, hf * 512 : (hf + 1) * 512],
                            start=True,
                            stop=True,
                        )
                        nc.vector.tensor_add(
                            xz[lt][:, hf * 512 : (hf + 1) * 512],
                            xz[lt][:, hf * 512 : (hf + 1) * 512],
                            pa,
                        )
                    # LayerNorm 1 -> z (in place over xz)
                    st6 = st_pool.tile([128, 2, 6], F32, name="st6", tag="st6")
                    nc.vector.bn_stats(st6[:, 0, :], xz[lt][:, 0:512])
                    nc.vector.bn_stats(st6[:, 1, :], xz[lt][:, 512:1024])
                    mv = st_pool.tile([128, 2], F32, name="mv", tag="mv")
                    nc.vector.bn_aggr(mv, st6)
                    sd = st_pool.tile([128, 1], F32, name="sd", tag="sd")
                    nc.scalar.activation(
                        sd, mv[:, 1:2], AF.Sqrt, scale=float(D) / (D - 1)
                    )
                    nc.vector.tensor_scalar_add(sd, sd, EPS)
                    iv = st_pool.tile([128, 1], F32, name="iv", tag="iv")
                    nc.vector.reciprocal(iv, sd)
                    nc.vector.tensor_scalar(
                        xz[lt], xz[lt], mv[:, 0:1], iv, ALU.subtract, ALU.mult
                    )
                    if not g1_trivial:
                        nc.vector.tensor_mul(xz[lt], xz[lt], g1_sb)
                        nc.vector.tensor_add(xz[lt], xz[lt], be1_sb)
                    # h^T via PE transpose, rounded to fp32r on copy
                    for d in range(DT):
                        pzt = ps_zt.tile([128, 128], F32, name="pzt", tag="pzt")
                        nc.tensor.transpose(
                            pzt, xz[lt][:, d * 128 : (d + 1) * 128], id_sb
                        )
                        if d % 2 == 0:
                            nc.scalar.activation(
                                ht_sb[:, d, l1 : l1 + 128], pzt, AF.Copy
                            )
                        else:
                            nc.vector.tensor_copy(ht_sb[:, d, l1 : l1 + 128], pzt)

                # conv + bias + leaky relu + residual + LayerNorm 2
                for lt in range(LTPC):
                    l1 = lt * 128
                    ya = ya_pool.tile([128, D], F32, name="ya", tag="ya")
                    for hf in range(2):
                        pc = ps_mm.tile([128, 512], F32, name="pc", tag="mm")
                        for d in range(DT):
                            nc.tensor.matmul(
                                pc,
                                ht_sb[:, d, l1 : l1 + 128],
                                wc_sb[:, d, hf * 512 : (hf + 1) * 512],
                                start=(d == 0),
                                stop=False,
                            )
                        nc.tensor.matmul(
                            pc,
                            ones_sb,
                            bc_sb[0:1, hf * 512 : (hf + 1) * 512],
                            start=False,
                            stop=True,
                        )
                        nc.scalar.activation(
                            ya[:, hf * 512 : (hf + 1) * 512], pc, AF.Lrelu, alpha=0.01
                        )
                    nc.vector.tensor_add(ya, ya, xz[lt])
                    st6b = st_pool.tile([128, 2, 6], F32, name="st6b", tag="st6b")
                    nc.vector.bn_stats(st6b[:, 0, :], ya[:, 0:512])
                    nc.vector.bn_stats(st6b[:, 1, :], ya[:, 512:1024])
                    mvb = st_pool.tile([128, 2], F32, name="mvb", tag="mvb")
                    nc.vector.bn_aggr(mvb, st6b)
                    sdb = st_pool.tile([128, 1], F32, name="sdb", tag="sdb")
                    nc.scalar.activation(
                        sdb, mvb[:, 1:2], AF.Sqrt, scale=float(D) / (D - 1)
                    )
                    nc.vector.tensor_scalar_add(sdb, sdb, EPS)
                    ivb = st_pool.tile([128, 1], F32, name="ivb", tag="ivb")
                    nc.vector.reciprocal(ivb, sdb)
                    nc.vector.tensor_scalar(
                        ya, ya, mvb[:, 0:1], ivb, ALU.subtract, ALU.mult
                    )
                    if not g2_trivial:
                        nc.vector.tensor_mul(ya, ya, g2_sb)
                        nc.vector.tensor_add(ya, ya, be2_sb)
                    nc.sync.dma_start(out_ap[l0 + l1 : l0 + l1 + 128, :], ya)
            if loop_cm is not None:
                loop_cm.__exit__(None, None, None)

    nc.compile()
    return nc


def _get_nc(g1_trivial, g2_trivial):
    key = (g1_trivial, g2_trivial)
    if key not in _CACHE:
        _CACHE[key] = _build(*key)
    return _CACHE[key]


def kernel(x, mask, W_Q, b_Q, C_K, C_V, g1, be1, Wc, bc, g2, be2):
    x = np.asarray(x, dtype=np.float32)
    mask = np.asarray(mask)
    W_Q = np.asarray(W_Q, dtype=np.float32)
    b_Q = np.asarray(b_Q, dtype=np.float32)
    C_K = np.asarray(C_K, dtype=np.float32)
    C_V = np.asarray(C_V, dtype=np.float32)
    g1 = np.asarray(g1, dtype=np.float32)
    be1 = np.asarray(be1, dtype=np.float32)
    Wc = np.asarray(Wc, dtype=np.float32)
    bc = np.asarray(bc, dtype=np.float32)
    g2 = np.asarray(g2, dtype=np.float32)
    be2 = np.asarray(be2, dtype=np.float32)

    g1_trivial = bool(np.all(g1 == 1.0) and np.all(be1 == 0.0))
    g2_trivial = bool(np.all(g2 == 1.0) and np.all(be2 == 0.0))
    nc = _get_nc(g1_trivial, g2_trivial)

    wq16 = np.ascontiguousarray(W_Q.T).astype(ml_dtypes.bfloat16)
    wcT = np.ascontiguousarray(Wc.T)
    ck16 = C_K.astype(ml_dtypes.bfloat16)
    cvT = np.ascontiguousarray(C_V.T)
    bq_t = np.ascontiguousarray(b_Q.reshape(DT, 128).T)
    bc_row = bc.reshape(1, D)
    ident = np.eye(128, dtype=np.float32)
    ones_row = np.ones((1, 128), dtype=np.float32)

    in_maps = []
    for b in range(B):
        m = {
            "x": np.ascontiguousarray(x[b]),
            "xt": np.ascontiguousarray(x[b].T.astype(ml_dtypes.bfloat16)),
            "wq": wq16,
            "wc": wcT,
            "ck": ck16,
            "cv": cvT,
            "bq": bq_t,
            "bc": bc_row,
            "maskf": np.ascontiguousarray(
                mask[b].astype(np.float32).reshape(L // 128, 128).T
            ),
            "ident": ident,
            "ones": ones_row,
        }
        if not g1_trivial:
            m["g1r"] = g1.reshape(1, D)
            m["be1r"] = be1.reshape(1, D)
        if not g2_trivial:
            m["g2r"] = g2.reshape(1, D)
            m["be2r"] = be2.reshape(1, D)
        in_maps.append(m)

    res = run_bass_kernel_spmd(nc, in_maps, core_ids=list(range(B)))
    return np.stack([res.results[b]["out"] for b in range(B)], axis=0)


# revision 7
# speedup vs baseline: 1.0541x; 1.0541x over previous
"""CoLA encoder layer on 8 Trainium2 NeuronCores, data-parallel over batch.

Per core (one batch element, L=4096 tokens, D=1024, ALPHA=64):
  Q^T = W_Q^T-major matmul (bf16), S = Q@C_K (bf16), softmax+mask,
  attn = A@C_V^T (fp32r), LayerNorm residual (fp32), pointwise conv
  (fp32r) + LeakyReLU, second LayerNorm residual. Token-major layout for
  LayerNorms/softmax; feature-major (transposed) operands for matmuls,
  produced on host (x^T) or on chip (z^T via PE transpose).
"""

import sys

for _p in ("/opt/trn_rl_repo",):
    if _p not in sys.path:
        sys.path.insert(0, _p)

from contextlib import ExitStack

import ml_dtypes
import numpy as np

import concourse.bass as bass
import concourse.tile as tile
from concourse import bacc, mybir
from concourse.bass_utils import run_bass_kernel_spmd

F32 = mybir.dt.float32
F32R = mybir.dt.float32r
BF16 = mybir.dt.bfloat16
AF = mybir.ActivationFunctionType
ALU = mybir.AluOpType
AX = mybir.AxisListType

B, L, D, ALPHA = 8, 4096, 1024, 64
EPS = 1e-6
INV_SQRT_D = 1.0 / 32.0
CH = 512           # tokens per chunk
NCH = L // CH      # 8 chunks
LTPC = CH // 128   # l-tiles per chunk
DT = D // 128      # feature tiles

_CACHE = {}


def _build(g1_trivial: bool, g2_trivial: bool, time_iters: int = 1):
    nc = bacc.Bacc("TRN2", target_bir_lowering=False, debug=False)

    x_d = nc.dram_tensor("x", [L, D], F32, kind="ExternalInput")
    xt_d = nc.dram_tensor("xt", [D, L], BF16, kind="ExternalInput")
    wq_d = nc.dram_tensor("wq", [D, D], BF16, kind="ExternalInput")
    wc_d = nc.dram_tensor("wc", [D, D], F32R, kind="ExternalInput")
    ck_d = nc.dram_tensor("ck", [D, ALPHA], BF16, kind="ExternalInput")
    cv_d = nc.dram_tensor("cv", [ALPHA, D], F32R, kind="ExternalInput")
    bq_d = nc.dram_tensor("bq", [128, DT], F32, kind="ExternalInput")
    bc_d = nc.dram_tensor("bc", [1, D], F32R, kind="ExternalInput")
    mask_d = nc.dram_tensor("maskf", [128, L // 128], F32, kind="ExternalInput")
    id_d = nc.dram_tensor("ident", [128, 128], F32, kind="ExternalInput")
    ones_d = nc.dram_tensor("ones", [1, 128], F32R, kind="ExternalInput")
    if not g1_trivial:
        g1_d = nc.dram_tensor("g1r", [1, D], F32, kind="ExternalInput")
        be1_d = nc.dram_tensor("be1r", [1, D], F32, kind="ExternalInput")
    if not g2_trivial:
        g2_d = nc.dram_tensor("g2r", [1, D], F32, kind="ExternalInput")
        be2_d = nc.dram_tensor("be2r", [1, D], F32, kind="ExternalInput")
    out_d = nc.dram_tensor("out", [L, D], F32, kind="ExternalOutput")

    x_ap = x_d.ap()
    xt_ap = xt_d.ap().rearrange("(t p) l -> p t l", p=128)
    out_ap = out_d.ap()

    with tile.TileContext(nc) as tc:
        with ExitStack() as ctx:
            wp = ctx.enter_context(tc.tile_pool(name="weights", bufs=1))
            xt_pool = ctx.enter_context(tc.tile_pool(name="xtp", bufs=2))
            qt_pool = ctx.enter_context(tc.tile_pool(name="qtp", bufs=2))
            ht_pool = ctx.enter_context(tc.tile_pool(name="htp", bufs=2))
            xz_pool = ctx.enter_context(tc.tile_pool(name="xzp", bufs=8))
            ya_pool = ctx.enter_context(tc.tile_pool(name="yap", bufs=6))
            a_pool = ctx.enter_context(tc.tile_pool(name="ap", bufs=8))
            st_pool = ctx.enter_context(tc.tile_pool(name="stp", bufs=12))
            # PSUM: mm512 (Q + z-transpose) 2 banks, mm1024 (attn + conv)
            # 2x2 banks, S 1 bank, A^T 1 bank = 8 banks total
            ps_mm = ctx.enter_context(tc.tile_pool(name="psmm", bufs=2, space="PSUM"))
            ps_big = ctx.enter_context(tc.tile_pool(name="psbig", bufs=2, space="PSUM"))
            ps_s = ctx.enter_context(tc.tile_pool(name="pss", bufs=1, space="PSUM"))
            ps_at = ctx.enter_context(tc.tile_pool(name="psat", bufs=1, space="PSUM"))

            wq_sb = wp.tile([128, DT, D], BF16)
            nc.sync.dma_start(wq_sb, wq_d.ap().rearrange("(t p) e -> p t e", p=128))
            wc_sb = wp.tile([128, DT, D], F32R)
            nc.sync.dma_start(wc_sb, wc_d.ap().rearrange("(t p) e -> p t e", p=128))
            ck_sb = wp.tile([128, DT, ALPHA], BF16)
            nc.sync.dma_start(ck_sb, ck_d.ap().rearrange("(t p) a -> p t a", p=128))
            cv_sb = wp.tile([ALPHA, D], F32R)
            nc.sync.dma_start(cv_sb, cv_d.ap())
            bq_sb = wp.tile([128, DT], F32)
            nc.sync.dma_start(bq_sb, bq_d.ap())
            bc_sb = wp.tile([1, D], F32R)
            nc.sync.dma_start(bc_sb, bc_d.ap())
            mask_sb = wp.tile([128, L // 128], F32)
            nc.sync.dma_start(mask_sb, mask_d.ap())
            id_sb = wp.tile([128, 128], F32)
            nc.sync.dma_start(id_sb, id_d.ap())
            ones_sb = wp.tile([1, 128], F32R)
            nc.sync.dma_start(ones_sb, ones_d.ap())
            if not g1_trivial:
                g1_sb = wp.tile([128, D], F32)
                nc.sync.dma_start(g1_sb, g1_d.ap().to_broadcast((128, D)))
                be1_sb = wp.tile([128, D], F32)
                nc.sync.dma_start(be1_sb, be1_d.ap().to_broadcast((128, D)))
            if not g2_trivial:
                g2_sb = wp.tile([128, D], F32)
                nc.sync.dma_start(g2_sb, g2_d.ap().to_broadcast((128, D)))
                be2_sb = wp.tile([128, D], F32)
                nc.sync.dma_start(be2_sb, be2_d.ap().to_broadcast((128, D)))

            loop_cm = (
                tc.For_i(0, time_iters, 1) if time_iters > 1 else None
            )
            if loop_cm is not None:
                loop_cm.__enter__()
            for c in range(NCH):
                l0 = c * CH
                xt_sb = xt_pool.tile([128, DT, CH], BF16, tag="xt")
                nc.sync.dma_start(xt_sb, xt_ap[:, :, l0 : l0 + CH])
                xz = []
                for lt in range(LTPC):
                    t = xz_pool.tile([128, D], F32, name=f"xz{lt}", tag="xz")
                    nc.sync.dma_start(
                        t, x_ap[l0 + lt * 128 : l0 + (lt + 1) * 128, :]
                    )
                    xz.append(t)

                # Q^T[e, l] per e-tile, bf16 inputs, +b_Q on PSUM copy
                qt_sb = qt_pool.tile([128, DT, CH], BF16, tag="qt")
                for e in range(DT):
                    pq = ps_mm.tile([128, CH], F32, name="pq", tag="mm512")
                    for d in range(DT):
                        nc.tensor.matmul(
                            pq,
                            wq_sb[:, d, e * 128 : (e + 1) * 128],
                            xt_sb[:, d, :],
                            start=(d == 0),
                            stop=(d == DT - 1),
                        )
                    nc.scalar.activation(
                        qt_sb[:, e, :], pq, AF.Identity, bias=bq_sb[:, e : e + 1]
                    )

                for lt in range(LTPC):
                    l1 = lt * 128
                    # S[l, a] then softmax
                    ps = ps_s.tile([128, ALPHA], F32, name="ps", tag="ps")
                    for e in range(DT):
                        nc.tensor.matmul(
                            ps,
                            qt_sb[:, e, l1 : l1 + 128],
                            ck_sb[:, e, :],
                            start=(e == 0),
                            stop=(e == DT - 1),
                        )
                    mx = st_pool.tile([128, 1], F32, name="mx", tag="mx")
                    nc.vector.reduce_max(mx, ps, axis=AX.X)
                    nm = st_pool.tile([128, 1], F32, name="nm", tag="nm")
                    nc.scalar.activation(nm, mx, AF.Copy, scale=-INV_SQRT_D)
                    ev = a_pool.tile([128, ALPHA], F32, name="ev", tag="ev")
                    sm = st_pool.tile([128, 1], F32, name="sm", tag="sm")
                    nc.scalar.activation(
                        ev, ps, AF.Exp, scale=INV_SQRT_D, bias=nm, accum_out=sm
                    )
                    rc = st_pool.tile([128, 1], F32, name="rc", tag="rc")
                    nc.vector.reciprocal(rc, sm)
                    r2 = st_pool.tile([128, 1], F32, name="r2", tag="r2")
                    li = c * LTPC + lt
                    nc.vector.tensor_mul(r2, rc, mask_sb[:, li : li + 1])
                    nc.vector.tensor_scalar_mul(ev, ev, r2)
                    # A^T
                    pat = ps_at.tile([ALPHA, 128], F32, name="pat", tag="pat")
                    nc.tensor.transpose(pat, ev, id_sb)
                    at = a_pool.tile([ALPHA, 128], F32R, name="at", tag="at")
                    nc.vector.tensor_copy(at, pat)
                    # attn + residual (single 2-bank PSUM tile, one DVE add)
                    pa = ps_big.tile([128, D], F32, name="pa", tag="mm1024")
                    for hf in range(2):
                        nc.tensor.matmul(
                            pa[:, hf * 512 : (hf + 1) * 512],
                            at,
                            cv_sb[:, hf * 512 : (hf + 1) * 512],
                            start=True,
                            stop=True,
                        )
                    nc.vector.tensor_add(xz[lt], xz[lt], pa)
                    # LayerNorm 1 -> z (in place over xz)
                    st6 = st_pool.tile([128, 2, 6], F32, name="st6", tag="st6")
                    nc.vector.bn_stats(st6[:, 0, :], xz[lt][:, 0:512])
                    nc.vector.bn_stats(st6[:, 1, :], xz[lt][:, 512:1024])
                    mv = st_pool.tile([128, 2], F32, name="mv", tag="mv")
                    nc.vector.bn_aggr(mv, st6)
                    sd = st_pool.tile([128, 1], F32, name="sd", tag="sd")
                    nc.scalar.activation(
                        sd, mv[:, 1:2], AF.Sqrt, scale=float(D) / (D - 1)
                    )
                    nc.vector.tensor_scalar_add(sd, sd, EPS)
                    iv = st_pool.tile([128, 1], F32, name="iv", tag="iv")
                    nc.vector.reciprocal(iv, sd)
                    nc.vector.tensor_scalar(
                        xz[lt], xz[lt], mv[:, 0:1], iv, ALU.subtract, ALU.mult
                    )
                    if not g1_trivial:
                        nc.vector.tensor_mul(xz[lt], xz[lt], g1_sb)
                        nc.vector.tensor_add(xz[lt], xz[lt], be1_sb)

                # h^T via PE transpose, d-outer so 4 l-tiles share one
                # PSUM bank and one PSUM->SBUF copy per d-tile
                ht_sb = ht_pool.tile([128, DT, CH], F32R, tag="ht")
                for d in range(DT):
                    pzt = ps_mm.tile([128, CH], F32, name="pzt", tag="mm512")
                    for lt in range(LTPC):
                        nc.tensor.transpose(
                            pzt[:, lt * 128 : (lt + 1) * 128],
                            xz[lt][:, d * 128 : (d + 1) * 128],
                            id_sb,
                        )
                    if d % 2 == 0:
                        nc.scalar.activation(ht_sb[:, d, :], pzt, AF.Copy)
                    else:
                        nc.vector.tensor_copy(ht_sb[:, d, :], pzt)

                # conv + bias + leaky relu + residual + LayerNorm 2
                for lt in range(LTPC):
                    l1 = lt * 128
                    ya = ya_pool.tile([128, D], F32, name="ya", tag="ya")
                    pc = ps_big.tile([128, D], F32, name="pc", tag="mm1024")
                    for hf in range(2):
                        pch = pc[:, hf * 512 : (hf + 1) * 512]
                        for d in range(DT):
                            nc.tensor.matmul(
                                pch,
                                ht_sb[:, d, l1 : l1 + 128],
                                wc_sb[:, d, hf * 512 : (hf + 1) * 512],
                                start=(d == 0),
                                stop=False,
                            )
                        nc.tensor.matmul(
                            pch,
                            ones_sb,
                            bc_sb[0:1, hf * 512 : (hf + 1) * 512],
                            start=False,
                            stop=True,
                        )
                    nc.scalar.activation(ya, pc, AF.Lrelu, alpha=0.01)
                    nc.gpsimd.tensor_add(ya, ya, xz[lt])
                    st6b = st_pool.tile([128, 2, 6], F32, name="st6b", tag="st6b")
                    nc.vector.bn_stats(st6b[:, 0, :], ya[:, 0:512])
                    nc.vector.bn_stats(st6b[:, 1, :], ya[:, 512:1024])
                    mvb = st_pool.tile([128, 2], F32, name="mvb", tag="mvb")
                    nc.vector.bn_aggr(mvb, st6b)
                    sdb = st_pool.tile([128, 1], F32, name="sdb", tag="sdb")
                    nc.scalar.activation(
                        sdb, mvb[:, 1:2], AF.Sqrt, scale=float(D) / (D - 1)
                    )
                    nc.vector.tensor_scalar_add(sdb, sdb, EPS)
                    ivb = st_pool.tile([128, 1], F32, name="ivb", tag="ivb")
                    nc.vector.reciprocal(ivb, sdb)
                    # out = (y - mean2)*inv2 on ACT: scale=inv2, bias=-mean2*inv2
                    nmi = st_pool.tile([128, 1], F32, name="nmi", tag="nmi")
                    nc.vector.tensor_scalar(
                        nmi, mvb[:, 0:1], -1.0, ivb, ALU.mult, ALU.mult
                    )
                    nc.scalar.activation(ya, ya, AF.Identity, bias=nmi, scale=ivb)
                    if not g2_trivial:
                        nc.vector.tensor_mul(ya, ya, g2_sb)
                        nc.vector.tensor_add(ya, ya, be2_sb)
                    nc.sync.dma_start(out_ap[l0 + l1 : l0 + l1 + 128, :], ya)
            if loop_cm is not None:
                loop_cm.__exit__(None, None, None)

    nc.compile()
    return nc


def _get_nc(g1_trivial, g2_trivial):
    key = (g1_trivial, g2_trivial)
    if key not in _CACHE:
        _CACHE[key] = _build(*key)
    return _CACHE[key]


def kernel(x, mask, W_Q, b_Q, C_K, C_V, g1, be1, Wc, bc, g2, be2):
    x = np.asarray(x, dtype=np.float32)
    mask = np.asarray(mask)
    W_Q = np.asarray(W_Q, dtype=np.float32)
    b_Q = np.asarray(b_Q, dtype=np.float32)
    C_K = np.asarray(C_K, dtype=np.float32)
    C_V = np.asarray(C_V, dtype=np.float32)
    g1 = np.asarray(g1, dtype=np.float32)
    be1 = np.asarray(be1, dtype=np.float32)
    Wc = np.asarray(Wc, dtype=np.float32)
    bc = np.asarray(bc, dtype=np.float32)
    g2 = np.asarray(g2, dtype=np.float32)
    be2 = np.asarray(be2, dtype=np.float32)

    g1_trivial = bool(np.all(g1 == 1.0) and np.all(be1 == 0.0))
    g2_trivial = bool(np.all(g2 == 1.0) and np.all(be2 == 0.0))
    nc = _get_nc(g1_trivial, g2_trivial)

    wq16 = np.ascontiguousarray(W_Q.T).astype(ml_dtypes.bfloat16)
    wcT = np.ascontiguousarray(Wc.T)
    ck16 = C_K.astype(ml_dtypes.bfloat16)
    cvT = np.ascontiguousarray(C_V.T)
    bq_t = np.ascontiguousarray(b_Q.reshape(DT, 128).T)
    bc_row = bc.reshape(1, D)
    ident = np.eye(128, dtype=np.float32)
    ones_row = np.ones((1, 128), dtype=np.float32)

    in_maps = []
    for b in range(B):
        m = {
            "x": np.ascontiguousarray(x[b]),
            "xt": np.ascontiguousarray(x[b].T.astype(ml_dtypes.bfloat16)),
            "wq": wq16,
            "wc": wcT,
            "ck": ck16,
            "cv": cvT,
            "bq": bq_t,
            "bc": bc_row,
            "maskf": np.ascontiguousarray(
                mask[b].astype(np.float32).reshape(L // 128, 128).T
            ),
            "ident": ident,
            "ones": ones_row,
        }
        if not g1_trivial:
            m["g1r"] = g1.reshape(1, D)
            m["be1r"] = be1.reshape(1, D)
        if not g2_trivial:
            m["g2r"] = g2.reshape(1, D)
            m["be2r"] = be2.reshape(1, D)
        in_maps.append(m)

    res = run_bass_kernel_spmd(nc, in_maps, core_ids=list(range(B)))
    return np.stack([res.results[b]["out"] for b in range(B)], axis=0)


# revision 13
# speedup vs baseline: 1.5966x; 1.5146x over previous
"""CoLA encoder layer on 8 Trainium2 NeuronCores, data-parallel over batch.

Per core (one batch element, L=4096 tokens, D=1024, ALPHA=64):
  Q^T = W_Q^T-major matmul (bf16), S = Q@C_K (bf16), softmax+mask,
  attn = A@C_V^T (fp32r), LayerNorm residual (fp32), pointwise conv
  (fp32r) + LeakyReLU, second LayerNorm residual. Token-major layout for
  LayerNorms/softmax; feature-major (transposed) operands for matmuls,
  produced on host (x^T) or on chip (z^T via PE transpose).
"""

import sys

for _p in ("/opt/trn_rl_repo",):
    if _p not in sys.path:
        sys.path.insert(0, _p)

from contextlib import ExitStack

import ml_dtypes
import numpy as np

import concourse.bass as bass
import concourse.tile as tile
from concourse import bacc, mybir
from concourse.bass_utils import run_bass_kernel_spmd

F32 = mybir.dt.float32
F32R = mybir.dt.float32r
BF16 = mybir.dt.bfloat16
AF = mybir.ActivationFunctionType
ALU = mybir.AluOpType
AX = mybir.AxisListType

B, L, D, ALPHA = 8, 4096, 1024, 64
EPS = 1e-6
INV_SQRT_D = 1.0 / 32.0
CH = 512           # tokens per chunk
NCH = L // CH      # 8 chunks
LTPC = CH // 128   # l-tiles per chunk
DT = D // 128      # feature tiles

_CACHE = {}


def _build(g1_trivial: bool, g2_trivial: bool, bq_trivial: bool = True, time_iters: int = 1):
    nc = bacc.Bacc("TRN2", target_bir_lowering=False, debug=False)

    x_d = nc.dram_tensor("x", [L, D], F32, kind="ExternalInput")
    xt_d = nc.dram_tensor("xt", [D, L], BF16, kind="ExternalInput")
    wq_d = nc.dram_tensor("wq", [D, D], BF16, kind="ExternalInput")
    wc_d = nc.dram_tensor("wc", [D, D], F32R, kind="ExternalInput")
    ck_d = nc.dram_tensor("ck", [D, ALPHA], BF16, kind="ExternalInput")
    cv_d = nc.dram_tensor("cv", [ALPHA, D], F32R, kind="ExternalInput")
    bq_d = nc.dram_tensor("bq", [128, DT], F32, kind="ExternalInput")
    bc_d = nc.dram_tensor("bc", [1, D], F32R, kind="ExternalInput")
    mask_d = nc.dram_tensor("maskf", [128, L // 128], F32, kind="ExternalInput")
    id_d = nc.dram_tensor("ident", [128, 128], F32, kind="ExternalInput")
    ones_d = nc.dram_tensor("ones", [1, 128], F32R, kind="ExternalInput")
    if not g1_trivial:
        g1_d = nc.dram_tensor("g1r", [1, D], F32, kind="ExternalInput")
        be1_d = nc.dram_tensor("be1r", [1, D], F32, kind="ExternalInput")
    if not g2_trivial:
        g2_d = nc.dram_tensor("g2r", [1, D], F32, kind="ExternalInput")
        be2_d = nc.dram_tensor("be2r", [1, D], F32, kind="ExternalInput")
    out_d = nc.dram_tensor("out", [L, D], F32, kind="ExternalOutput")

    x_ap = x_d.ap()
    xt_ap = xt_d.ap().rearrange("(t p) l -> p t l", p=128)
    out_ap = out_d.ap()

    with tile.TileContext(nc) as tc:
        with ExitStack() as ctx:
            wp = ctx.enter_context(tc.tile_pool(name="weights", bufs=1))
            xt_pool = ctx.enter_context(tc.tile_pool(name="xtp", bufs=2))
            qt_pool = ctx.enter_context(tc.tile_pool(name="qtp", bufs=2))
            ht_pool = ctx.enter_context(tc.tile_pool(name="htp", bufs=2))
            xz_pool = ctx.enter_context(tc.tile_pool(name="xzp", bufs=8))
            ya_pool = ctx.enter_context(tc.tile_pool(name="yap", bufs=6))
            a_pool = ctx.enter_context(tc.tile_pool(name="ap", bufs=8))
            st_pool = ctx.enter_context(tc.tile_pool(name="stp", bufs=12))
            sq_pool = ctx.enter_context(tc.tile_pool(name="sqp", bufs=2))
            # PSUM: mm512 (Q + z-transpose) 2 banks, mm1024 (attn + conv)
            # 2x2 banks, S 1 bank, A^T 1 bank = 8 banks total
            ps_mm = ctx.enter_context(tc.tile_pool(name="psmm", bufs=2, space="PSUM"))
            ps_big = ctx.enter_context(tc.tile_pool(name="psbig", bufs=2, space="PSUM"))
            ps_s = ctx.enter_context(tc.tile_pool(name="pss", bufs=1, space="PSUM"))
            ps_at = ctx.enter_context(tc.tile_pool(name="psat", bufs=1, space="PSUM"))

            wq_sb = wp.tile([128, DT, D], BF16)
            nc.sync.dma_start(wq_sb, wq_d.ap().rearrange("(t p) e -> p t e", p=128))
            ck_sb = wp.tile([128, DT, ALPHA], BF16)
            nc.sync.dma_start(ck_sb, ck_d.ap().rearrange("(t p) a -> p t a", p=128))
            cv_sb = wp.tile([ALPHA, D], F32R)
            nc.sync.dma_start(cv_sb, cv_d.ap())
            wc_sb = wp.tile([128, DT, D], F32R)
            nc.sync.dma_start(wc_sb, wc_d.ap().rearrange("(t p) e -> p t e", p=128))
            bq_sb = wp.tile([128, DT], F32)
            nc.sync.dma_start(bq_sb, bq_d.ap())
            bc_sb = wp.tile([1, D], F32R)
            nc.sync.dma_start(bc_sb, bc_d.ap())
            mask_sb = wp.tile([128, L // 128], F32)
            nc.sync.dma_start(mask_sb, mask_d.ap())
            id_sb = wp.tile([128, 128], F32)
            nc.sync.dma_start(id_sb, id_d.ap())
            ones_sb = wp.tile([1, 128], F32R)
            nc.sync.dma_start(ones_sb, ones_d.ap())
            if not g1_trivial:
                g1_sb = wp.tile([128, D], F32)
                nc.sync.dma_start(g1_sb, g1_d.ap().to_broadcast((128, D)))
                be1_sb = wp.tile([128, D], F32)
                nc.sync.dma_start(be1_sb, be1_d.ap().to_broadcast((128, D)))
            if not g2_trivial:
                g2_sb = wp.tile([128, D], F32)
                nc.sync.dma_start(g2_sb, g2_d.ap().to_broadcast((128, D)))
                be2_sb = wp.tile([128, D], F32)
                nc.sync.dma_start(be2_sb, be2_d.ap().to_broadcast((128, D)))

            def frontend(c):
                """DMA, Q^T, S, softmax, attn, residual, LN1 -> z in xz."""
                l0 = c * CH
                xt_sb = xt_pool.tile([128, DT, CH], BF16, tag="xt")
                nc.sync.dma_start(xt_sb, xt_ap[:, :, l0 : l0 + CH])
                xz = []
                for lt in range(LTPC):
                    t = xz_pool.tile([128, D], F32, name=f"xz{lt}", tag="xz")
                    nc.sync.dma_start(
                        t, x_ap[l0 + lt * 128 : l0 + (lt + 1) * 128, :]
                    )
                    xz.append(t)

                # Q^T[e, l] per e-tile, bf16 inputs, +b_Q on PSUM copy
                qt_sb = qt_pool.tile([128, DT, CH], BF16, tag="qt")
                for e in range(DT):
                    pq = ps_mm.tile([128, CH], F32, name="pq", tag="mm512")
                    for d in range(DT):
                        nc.tensor.matmul(
                            pq,
                            wq_sb[:, d, e * 128 : (e + 1) * 128],
                            xt_sb[:, d, :],
                            start=(d == 0),
                            stop=(d == DT - 1),
                        )
                    if bq_trivial:
                        if e % 2 == 0:
                            nc.scalar.activation(qt_sb[:, e, :], pq, AF.Copy)
                        else:
                            nc.vector.tensor_copy(qt_sb[:, e, :], pq)
                    else:
                        nc.vector.tensor_scalar_add(
                            qt_sb[:, e, :], pq, bq_sb[:, e : e + 1]
                        )

                # S for all 4 l-tiles into one PSUM bank [128, 4*64]
                ps = ps_s.tile([128, LTPC, ALPHA], F32, name="ps", tag="ps")
                for lt in range(LTPC):
                    l1 = lt * 128
                    for e in range(DT):
                        nc.tensor.matmul(
                            ps[:, lt, :],
                            qt_sb[:, e, l1 : l1 + 128],
                            ck_sb[:, e, :],
                            start=(e == 0),
                            stop=(e == DT - 1),
                        )
                # softmax, batched over the chunk; logits are tiny (|S|/32
                # < 0.25) so the max-subtraction is unnecessary
                ev4 = a_pool.tile([128, LTPC, ALPHA], F32, name="ev4", tag="ev4")
                nc.scalar.activation(ev4, ps, AF.Exp, scale=INV_SQRT_D)
                sm4 = st_pool.tile([128, LTPC], F32, name="sm4", tag="sm4")
                nc.vector.reduce_sum(sm4, ev4, axis=AX.X)
                rc4 = st_pool.tile([128, LTPC], F32, name="rc4", tag="rc4")
                nc.vector.reciprocal(rc4, sm4)
                r24 = st_pool.tile([128, LTPC], F32, name="r24", tag="r24")
                nc.vector.tensor_mul(
                    r24, rc4, mask_sb[:, c * LTPC : (c + 1) * LTPC]
                )
                r2b = bass.AP(
                    tensor=r24.tensor,
                    offset=r24.offset,
                    ap=[r24.ap[0], r24.ap[1], [0, ALPHA]],
                )
                nc.vector.tensor_mul(ev4, ev4, r2b)

                for lt in range(LTPC):
                    # A^T
                    pat = ps_at.tile([ALPHA, 128], F32, name="pat", tag="pat")
                    nc.tensor.transpose(pat, ev4[:, lt, :], id_sb)
                    at = a_pool.tile([ALPHA, 128], F32R, name="at", tag="at")
                    nc.vector.tensor_copy(at, pat)
                    # attn + residual; accum_out gives sum(r) for LN1
                    pa = ps_big.tile([128, D], F32, name="pa", tag="mm1024")
                    for hf in range(2):
                        nc.tensor.matmul(
                            pa[:, hf * 512 : (hf + 1) * 512],
                            at,
                            cv_sb[:, hf * 512 : (hf + 1) * 512],
                            start=True,
                            stop=True,
                        )
                    sum1 = st_pool.tile([128, LTPC], F32, name="sum1", tag="sum1") \
                        if lt == 0 else sum1
                    nc.vector.scalar_tensor_tensor(
                        xz[lt], xz[lt], 0.0, pa, ALU.add, ALU.add,
                        accum_out=sum1[:, lt : lt + 1],
                    )
                    ssq1 = st_pool.tile([128, LTPC], F32, name="ssq1", tag="ssq1") \
                        if lt == 0 else ssq1
                    sq = sq_pool.tile([128, D], F32, name="sq", tag="sq")
                    nc.scalar.activation(
                        sq, xz[lt], AF.Square, accum_out=ssq1[:, lt : lt + 1]
                    )

                # LN1 stats, batched [128, 4]: mean = sum/D,
                # var = (ssq - D*mean^2)/(D-1), inv = 1/(sqrt(var)+eps)
                mean1 = st_pool.tile([128, LTPC], F32, name="mean1", tag="mean1")
                nc.vector.tensor_scalar_mul(mean1, sum1, 1.0 / D)
                var1 = st_pool.tile([128, LTPC], F32, name="var1", tag="var1")
                nc.vector.tensor_mul(var1, mean1, mean1)
                nc.vector.scalar_tensor_tensor(
                    var1, var1, -float(D), ssq1, ALU.mult, ALU.add
                )
                sd1 = st_pool.tile([128, LTPC], F32, name="sd1", tag="sd1")
                nc.scalar.activation(sd1, var1, AF.Sqrt, scale=1.0 / (D - 1))
                nc.vector.tensor_scalar_add(sd1, sd1, EPS)
                iv1 = st_pool.tile([128, LTPC], F32, name="iv1", tag="iv1")
                nc.vector.reciprocal(iv1, sd1)
                for lt in range(LTPC):
                    nc.vector.tensor_scalar(
                        xz[lt], xz[lt], mean1[:, lt : lt + 1],
                        iv1[:, lt : lt + 1], ALU.subtract, ALU.mult,
                    )
                    if not g1_trivial:
                        nc.vector.tensor_mul(xz[lt], xz[lt], g1_sb)
                        nc.vector.tensor_add(xz[lt], xz[lt], be1_sb)
                return xz

            def backend(c, xz):
                """z^T transpose, conv, LeakyReLU, residual, LN2, DMA out."""
                l0 = c * CH
                # h^T via PE transpose, d-outer so 4 l-tiles share one
                # PSUM bank and one PSUM->SBUF copy per d-tile
                ht_sb = ht_pool.tile([128, DT, CH], F32R, tag="ht")
                for d in range(DT):
                    pzt = ps_mm.tile([128, CH], F32, name="pzt", tag="mm512")
                    for lt in range(LTPC):
                        nc.tensor.transpose(
                            pzt[:, lt * 128 : (lt + 1) * 128],
                            xz[lt][:, d * 128 : (d + 1) * 128],
                            id_sb,
                        )
                    if d % 2 == 0:
                        nc.scalar.activation(ht_sb[:, d, :], pzt, AF.Copy)
                    else:
                        nc.vector.tensor_copy(ht_sb[:, d, :], pzt)

                # conv + bias + leaky relu + residual + LayerNorm 2
                sum2 = st_pool.tile([128, LTPC], F32, name="sum2", tag="sum2")
                ssq2 = st_pool.tile([128, LTPC], F32, name="ssq2", tag="ssq2")
                yas = []
                for lt in range(LTPC):
                    l1 = lt * 128
                    ya = ya_pool.tile([128, D], F32, name="ya", tag="ya")
                    yas.append(ya)
                    pc = ps_big.tile([128, D], F32, name="pc", tag="mm1024")
                    for hf in range(2):
                        pch = pc[:, hf * 512 : (hf + 1) * 512]
                        for d in range(DT):
                            nc.tensor.matmul(
                                pch,
                                ht_sb[:, d, l1 : l1 + 128],
                                wc_sb[:, d, hf * 512 : (hf + 1) * 512],
                                start=(d == 0),
                                stop=False,
                            )
                        nc.tensor.matmul(
                            pch,
                            ones_sb,
                            bc_sb[0:1, hf * 512 : (hf + 1) * 512],
                            start=False,
                            stop=True,
                        )
                    nc.scalar.activation(ya, pc, AF.Lrelu, alpha=0.01)
                    nc.vector.scalar_tensor_tensor(
                        ya, ya, 0.0, xz[lt], ALU.add, ALU.add,
                        accum_out=sum2[:, lt : lt + 1],
                    )
                    sq2 = sq_pool.tile([128, D], F32, name="sq2", tag="sq")
                    nc.scalar.activation(
                        sq2, ya, AF.Square, accum_out=ssq2[:, lt : lt + 1]
                    )

                mean2 = st_pool.tile([128, LTPC], F32, name="mean2", tag="mean2")
                nc.vector.tensor_scalar_mul(mean2, sum2, 1.0 / D)
                var2 = st_pool.tile([128, LTPC], F32, name="var2", tag="var2")
                nc.vector.tensor_mul(var2, mean2, mean2)
                nc.vector.scalar_tensor_tensor(
                    var2, var2, -float(D), ssq2, ALU.mult, ALU.add
                )
                sd2 = st_pool.tile([128, LTPC], F32, name="sd2", tag="sd2")
                nc.scalar.activation(sd2, var2, AF.Sqrt, scale=1.0 / (D - 1))
                nc.vector.tensor_scalar_add(sd2, sd2, EPS)
                iv2 = st_pool.tile([128, LTPC], F32, name="iv2", tag="iv2")
                nc.vector.reciprocal(iv2, sd2)
                for lt in range(LTPC):
                    ya = yas[lt]
                    nc.vector.tensor_scalar(
                        ya, ya, mean2[:, lt : lt + 1], iv2[:, lt : lt + 1],
                        ALU.subtract, ALU.mult,
                    )
                    if not g2_trivial:
                        nc.vector.tensor_mul(ya, ya, g2_sb)
                        nc.vector.tensor_add(ya, ya, be2_sb)
                    nc.sync.dma_start(
                        out_ap[l0 + lt * 128 : l0 + (lt + 1) * 128, :], ya
                    )

            # one-chunk software skew: emit chunk c+1's frontend before
            # chunk c's backend so next-chunk Q matmuls fill the PE gap
            # left by the LN1 chain
            loop_cm = (
                tc.For_i(0, time_iters, 1) if time_iters > 1 else None
            )
            if loop_cm is not None:
                loop_cm.__enter__()
            prev = None
            for c in range(NCH):
                xz_c = frontend(c)
                if prev is not None:
                    backend(*prev)
                prev = (c, xz_c)
            backend(*prev)
            if loop_cm is not None:
                loop_cm.__exit__(None, None, None)

    nc.compile()
    return nc


def _get_nc(g1_trivial, g2_trivial, bq_trivial):
    key = (g1_trivial, g2_trivial, bq_trivial)
    if key not in _CACHE:
        _CACHE[key] = _build(*key)
    return _CACHE[key]


def kernel(x, mask, W_Q, b_Q, C_K, C_V, g1, be1, Wc, bc, g2, be2):
    x = np.asarray(x, dtype=np.float32)
    mask = np.asarray(mask)
    W_Q = np.asarray(W_Q, dtype=np.float32)
    b_Q = np.asarray(b_Q, dtype=np.float32)
    C_K = np.asarray(C_K, dtype=np.float32)
    C_V = np.asarray(C_V, dtype=np.float32)
    g1 = np.asarray(g1, dtype=np.float32)
    be1 = np.asarray(be1, dtype=np.float32)
    Wc = np.asarray(Wc, dtype=np.float32)
    bc = np.asarray(bc, dtype=np.float32)
    g2 = np.asarray(g2, dtype=np.float32)
    be2 = np.asarray(be2, dtype=np.float32)

    g1_trivial = bool(np.all(g1 == 1.0) and np.all(be1 == 0.0))
    g2_trivial = bool(np.all(g2 == 1.0) and np.all(be2 == 0.0))
    bq_trivial = bool(np.all(b_Q == 0.0))
    nc = _get_nc(g1_trivial, g2_trivial, bq_trivial)

    wq16 = np.ascontiguousarray(W_Q.T).astype(ml_dtypes.bfloat16)
    wcT = np.ascontiguousarray(Wc.T)
    ck16 = C_K.astype(ml_dtypes.bfloat16)
    cvT = np.ascontiguousarray(C_V.T)
    bq_t = np.ascontiguousarray(b_Q.reshape(DT, 128).T)
    bc_row = bc.reshape(1, D)
    ident = np.eye(128, dtype=np.float32)
    ones_row = np.ones((1, 128), dtype=np.float32)

    in_maps = []
    for b in range(B):
        m = {
            "x": np.ascontiguousarray(x[b]),
            "xt": np.ascontiguousarray(x[b].T.astype(ml_dtypes.bfloat16)),
            "wq": wq16,
            "wc": wcT,
            "ck": ck16,
            "cv": cvT,
            "bq": bq_t,
            "bc": bc_row,
            "maskf": np.ascontiguousarray(
                mask[b].astype(np.float32).reshape(L // 128, 128).T
            ),
            "ident": ident,
            "ones": ones_row,
        }
        if not g1_trivial:
            m["g1r"] = g1.reshape(1, D)
            m["be1r"] = be1.reshape(1, D)
        if not g2_trivial:
            m["g2r"] = g2.reshape(1, D)
            m["be2r"] = be2.reshape(1, D)
        in_maps.append(m)

    res = run_bass_kernel_spmd(nc, in_maps, core_ids=list(range(B)))
    return np.stack([res.results[b]["out"] for b in range(B)], axis=0)
